# revision 45
# baseline (speedup 1.0000x reference)
"""Multi-head causal attention (B=2, S=2048, D=1024, H=16) on 8 TRN2 NeuronCores.

Sharding: batch*head parallel. Core c handles batch b = c//4 and the 4
heads h in [4*(c%4), 4*(c%4)+4). Each core computes its heads' Q/K/V
projections (column-parallel), causal softmax attention, and its partial
row-parallel output projection; the host sums the 4 partial outputs per
batch (the AllReduce of row-parallel tensor parallelism).

On-device layout: everything is kept "transposed" (feature-major) so
every matmul contracts along the partition dimension:
  scoresT[k,q] = K Q^T      (per head, 128-row k-tiles x 512-col q-tiles)
  P^T = exp(scoresT/8)      (diagonal blocks column-restricted; the
                             straddling 128x128 triangle is zeroed by a
                             DVE multiply with a 0/1 triangle tile)
  outT[d,q]   = sum_k V[k,d] P^T[k,q]   (PSUM-accumulated over k-tiles)
  sums[q]     = sum_k P^T[k,q]          (ones-vector matmul, col-packed)
  y[q,e]     += sum_hd outT_norm[hd,q] * w_oT[hd,e]
Softmax skips the max-subtraction: scores ~ N(0,1), so exp never
overflows fp32; fully-masked columns are simply never computed.

x / q,k,v weights / attention / outproj all run in bf16 (measured
~3.5e-3 max rel err vs the f32 reference, tolerance 2e-2). x is
host-pre-tiled to [128, 8*2048] (feature-major e-tiles side by side) and
held resident in SBUF, so Q/K/V projections all read the same resident
tiles (no strided V re-load). Constant tiles (ones / 0-1 triangle) are
generated on the idle GPSIMD engine (iota / affine_select) instead of
DMAed. Input DMAs are split ~0.5MB per queue and issued from both the
SP and Activation hardware DGE sequencers to halve issue serialization.

The PE HAM clock gate starts at half clock and only reaches full clock
after ~3.4us of gapless PE activity; a dummy warm-up pack (results never
read) runs while the x DMAs stream.
"""

import numpy as np

D_MODEL = 1024
N_HEADS = 16
D_K = 64
B, S = 2, 2048
N_CORES = 8
HPC = 4            # heads per core
KT = S // 128      # 16 k-tiles
QT = S // 512      # 4 q-tiles
ET = D_MODEL // 128  # 8 e-tiles (contraction tiles for projections)

_PROG_CACHE = {}


def _build_program():
    import concourse.bacc as bacc_mod
    import concourse.mybir as mybir
    import concourse.tile as tile

    f32 = mybir.dt.float32
    f32r = mybir.dt.float32r
    bf16 = mybir.dt.bfloat16
    Exp = mybir.ActivationFunctionType.Exp

    nc = bacc_mod.Bacc(
        "TRN2", target_bir_lowering=False, debug=False, num_devices=N_CORES
    )

    xq = nc.dram_tensor("xq", [128, ET * S], bf16, kind="ExternalInput").ap()
    xk = nc.dram_tensor("xk", [128, ET * S], bf16, kind="ExternalInput").ap()
    xv = nc.dram_tensor("xv", [128, ET * S], bf16, kind="ExternalInput").ap()
    wq = nc.dram_tensor("wq", [128, ET * 256], bf16, kind="ExternalInput").ap()
    wk = nc.dram_tensor("wk", [128, ET * 256], bf16, kind="ExternalInput").ap()
    wv = nc.dram_tensor("wv", [128, ET * 256], bf16, kind="ExternalInput").ap()
    wo = nc.dram_tensor("wo", [256, D_MODEL], bf16, kind="ExternalInput").ap()
    y = nc.dram_tensor("y", [S, D_MODEL], bf16, kind="ExternalOutput").ap()

    with (
        tile.TileContext(nc) as tc,
        nc.allow_low_precision("bf16 attention"),
        tc.tile_pool(name="persist", bufs=1) as pp,
    ):
        # ---- persistent SBUF tiles ----
        def persist(shape, dtype, name):
            return pp.tile(shape, dtype, name=name, tag=name)

        xq_sb = persist([128, ET * S], bf16, "xq_sb")
        xk_sb = persist([128, ET * S], bf16, "xk_sb")
        xv_sb = persist([128, ET * S], bf16, "xv_sb")
        wq_sb = persist([128, ET * 256], bf16, "wq_sb")
        wk_sb = persist([128, ET * 256], bf16, "wk_sb")
        wv_sb = persist([128, ET * 256], bf16, "wv_sb")
        wo_sb = [persist([128, D_MODEL], bf16, f"wo_sb{p}") for p in range(2)]
        gen_sb = persist([128, 512], bf16, "gen_sb")     # all-ones (iota)
        tri_sb = persist([128, 256], bf16, "tri_sb")     # 0/1 upper-tri x2
        onesf_sb = persist([128, 64], f32r, "onesf_sb")  # ones (bc lhsT)
        qt_sb = [persist([128, S], bf16, f"qt_sb{p}") for p in range(2)]
        kt_sb = [persist([128, S], bf16, f"kt_sb{p}") for p in range(2)]
        v_sb = [persist([128, 260], bf16, f"v_sb{i}") for i in range(KT)]
        outt_sb = [persist([128, S], bf16, f"outt_sb{p}") for p in range(2)]

        # ---- GPSIMD-generated constants (no DMA dependencies) ----
        nc.gpsimd.iota(
            gen_sb[:], pattern=[[0, 512]], base=1, channel_multiplier=0,
            allow_small_or_imprecise_dtypes=True,
        )
        nc.vector.tensor_copy(onesf_sb[:], gen_sb[:, 0:64])
        # tri[p, a*128 + c] = 1.0 if p <= c else 0.0  (keep where col-row >= 0)
        nc.gpsimd.affine_select(
            tri_sb[:].rearrange("p (a c) -> p a c", a=2),
            gen_sb[:, 0:256].rearrange("p (a c) -> p a c", a=2),
            pattern=[[0, 2], [1, 128]],
            compare_op=mybir.AluOpType.is_ge,
            fill=0.0,
            base=0,
            channel_multiplier=-1,
        )

        # ---- input DMAs ----
        # Weights ride the Activation engine's DGE queues (early, small, and
        # done before the first exp needs the Act sequencer). The x stream
        # is SP-issued in k-column chunks, xv first within each chunk so
        # the V/Q/K projections of chunk 0 can start ~20us in while later
        # chunks land.
        for w_dram, w_tile in ((wq, wq_sb), (wk, wk_sb), (wv, wv_sb)):
            for h in range(2):
                cs = slice(h * ET * 128, (h + 1) * ET * 128)
                nc.scalar.dma_start(out=w_tile[:, cs], in_=w_dram[:, cs])
        for p in range(2):
            nc.scalar.dma_start(
                out=wo_sb[p][:], in_=wo[p * 128 : (p + 1) * 128, :]
            )

        def kchunk(ap, kc, eh):
            return ap.rearrange("p (t k) -> p t k", t=ET)[
                :, eh * 2 : (eh + 1) * 2, kc * 512 : (kc + 1) * 512
            ]

        for kc in range(4):
            for x_dram, x_sb in ((xv, xv_sb), (xq, xq_sb), (xk, xk_sb)):
                for eh in range(4):
                    nc.sync.dma_start(
                        out=kchunk(x_sb[:], kc, eh), in_=kchunk(x_dram, kc, eh)
                    )

        # ---- phase C+D: attention with interleaved output projection ----
        # One head-pair per pass (pr = 0, 1). Per (pr, j): score tiles are
        # [128, 1024] head-pair PSUM tiles; ONE exp per round (column-
        # restricted on diagonal blocks). attnV accumulates into a
        # [65, 1024] pair tile (row 64 = sum of exp via the ones column of
        # v_sb). Normalization of q-block j-1 is emitted lazily inside
        # block j; a staging copy releases its ps_out PSUM early so the
        # next block's attnV is not blocked. The output projection of
        # block j-1 runs as dense filler inside the pr=1 pass.
        with (
            tc.tile_pool(name="psS", bufs=2, space="PSUM") as psS,
            tc.tile_pool(name="psO", bufs=1, space="PSUM") as psO,
            tc.tile_pool(name="psP", bufs=2, space="PSUM") as psP,
            tc.tile_pool(name="et", bufs=6) as etp,
            tc.tile_pool(name="bcsb", bufs=3) as bcp,
            tc.tile_pool(name="rcsb", bufs=3) as rcp,
            tc.tile_pool(name="ysb", bufs=3) as ysbp,
        ):
            tri3 = tri_sb[:].rearrange("p (a c) -> p a c", a=2)

            # constant exp-sum ones columns of the V tiles (written once)
            for i in range(KT):
                nc.vector.tensor_copy(
                    v_sb[i][:].rearrange("p (h c) -> p h c", c=65)[:, :, 64:65],
                    gen_sb[:, 0:4].rearrange("p (h c) -> p h c", c=1),
                )

            def et3(t):
                return t[:].rearrange("p (h q) -> p h q", h=2)

            # ---- chunked projections (shared [128,512] PSUM ring) ----
            def emit_v_tile(i):
                psv = psP.tile([128, 512], f32, name=f"psv_{i}", tag="pp")
                for e in range(ET):
                    nc.tensor.matmul(
                        psv[:, 0:256],
                        xv_sb[:, e * S + i * 128 : e * S + (i + 1) * 128],
                        wv_sb[:, e * 256 : (e + 1) * 256],
                        start=(e == 0),
                        stop=(e == ET - 1),
                    )
                nc.scalar.copy(
                    v_sb[i][:].rearrange("p (h c) -> p h c", c=65)[:, :, 0:64],
                    psv[:, 0:256].rearrange("p (h d) -> p h d", d=64),
                )

            def emit_qk_chunk(which, m, kc):
                x_sb, w_tile, dst = (
                    (xq_sb, wq_sb, qt_sb) if which == "q" else (xk_sb, wk_sb, kt_sb)
                )
                psc = psP.tile(
                    [128, 512], f32, name=f"psc_{which}{m}_{kc}", tag="pp"
                )
                for e in range(ET):
                    nc.tensor.matmul(
                        psc[:],
                        w_tile[:, e * 256 + m * 128 : e * 256 + (m + 1) * 128],
                        x_sb[:, e * S + kc * 512 : e * S + (kc + 1) * 512],
                        start=(e == 0),
                        stop=(e == ET - 1),
                    )
                nc.scalar.copy(dst[m][:, kc * 512 : (kc + 1) * 512], psc[:])

            # PE warm-up: dummy matmuls (results never read) walk the HAM
            # clock gate up to full speed before the first projections.
            wt = psP.tile([128, 512], f32, name="warm", tag="pp")
            for w in range(20):
                nc.tensor.matmul(
                    wt[:], gen_sb[:, 0:128], gen_sb[:, 0:512],
                    start=True, stop=True,
                )

            # block-j0 prologue; later chunks go through the filler queue
            for i in range(4):
                emit_v_tile(i)
            emit_qk_chunk("q", 0, 0)
            emit_qk_chunk("k", 0, 0)
            fillers = []  # (needed_by_block, closure)
            for kc in range(1, 4):
                for i in range(4 * kc, 4 * kc + 4):
                    fillers.append((kc, ("v", i)))
                fillers.append((kc, ("q", 0, kc)))
                fillers.append((kc, ("k", 0, kc)))
            for kc in range(4):
                fillers.append((4, ("q", 1, kc)))
                fillers.append((4, ("k", 1, kc)))

            def pop_filler():
                kind = fillers.pop(0)[1]
                if kind[0] == "v":
                    emit_v_tile(kind[1])
                else:
                    emit_qk_chunk(*kind)

            def emit_outproj_chunk(m, n):
                psy = psP.tile([128, 512], f32, name=f"psy_{m}_{n}", tag="pp")
                for p in range(2):
                    nc.tensor.matmul(
                        psy[:],
                        outt_sb[p][:, m * 128 : (m + 1) * 128],
                        wo_sb[p][:, n * 512 : (n + 1) * 512],
                        start=(p == 0),
                        stop=(p == 1),
                    )
                y_sb = ysbp.tile([128, 512], bf16, name=f"y_sb_{m}_{n}", tag="ysb")
                nc.vector.tensor_copy(y_sb[:], psy[:])
                for q in range(2):
                    nc.sync.dma_start(
                        out=y[
                            m * 128 : (m + 1) * 128,
                            n * 512 + q * 256 : n * 512 + (q + 1) * 256,
                        ],
                        in_=y_sb[:, q * 256 : (q + 1) * 256],
                    )

            def emit_normalize(pr, jj, ps_out_prev):
                qsj = slice(jj * 512, (jj + 1) * 512)
                ssb = rcp.tile([33, 512], f32, name=f"ssb_{pr}_{jj}", tag="ssb")
                for hh in range(2):
                    nc.vector.tensor_copy(
                        ssb[32 * hh : 32 * hh + 1, :],
                        ps_out_prev[64:65, 512 * hh : 512 * (hh + 1)],
                    )
                # staging copy releases ps_out for the next block's attnV
                stg = rcp.tile([64, 1024], f32, name=f"stg_{pr}_{jj}", tag="stg")
                nc.vector.tensor_copy(stg[:], ps_out_prev[0:64, :])
                rc32 = rcp.tile([33, 512], f32, name=f"rc32_{pr}_{jj}", tag="rc32")
                nc.vector.reciprocal_approx_fast(out=rc32[:], in_=ssb[:])
                rc = rcp.tile([33, 512], f32r, name=f"rc_{pr}_{jj}", tag="rc")
                nc.vector.tensor_copy(rc[:], rc32[:])
                for hh in range(2):
                    bch = psP.tile(
                        [128, 512], f32, name=f"ps_bc_{pr}_{jj}_{hh}", tag="pp"
                    )
                    nc.tensor.matmul(
                        bch[0:64, :],
                        onesf_sb[32 * hh : 32 * hh + 1, 0:64],
                        rc[32 * hh : 32 * hh + 1, :],
                        start=True,
                        stop=True,
                        tile_position=(32 * hh, 0),
                    )
                    bc_sb = bcp.tile(
                        [64, 512], f32, name=f"bc_sb_{pr}_{jj}_{hh}", tag="bc"
                    )
                    nc.vector.tensor_copy(bc_sb[:], bch[0:64, :])
                    nc.vector.tensor_mul(
                        outt_sb[pr][64 * hh : 64 * hh + 64, qsj],
                        stg[:, 512 * hh : 512 * (hh + 1)],
                        bc_sb[:],
                    )

            pending_norms = []  # (pr, j, ps_out) awaiting lazy normalize
            ready_out = []      # j's with both norms done, awaiting outproj
            for pr in range(2):
                # ascending j in BOTH passes: pr0 tracks the k-chunked DMA
                # arrival; pr1's early (small) blocks finish their
                # normalizes first so outproj chunks drip out through the
                # larger later blocks instead of piling into a serial drain.
                j_order = range(QT)
                for j in j_order:
                    n_i = 4 * j + 4
                    # projections this block depends on (flush leftovers)
                    lim = j if pr == 0 else 5
                    while fillers and fillers[0][0] <= lim:
                        pop_filler()
                    ps_out = psO.tile(
                        [65, 1024], f32, name=f"ps_out_{pr}_{j}", tag="o"
                    )
                    prev_et = None
                    prev_i = -1
                    prev_lo = 0
                    for i in range(n_i):
                        diag = i >= 4 * j
                        r = i - 4 * j
                        lo = 128 * r if diag else 0
                        pss = psS.tile(
                            [128, 1024], f32, name=f"ps_s{pr}_{j}_{i}", tag="s"
                        )
                        for hh in range(2):
                            hp = slice(64 * hh, 64 * hh + 64)
                            nc.tensor.matmul(
                                pss[:, 512 * hh + lo : 512 * (hh + 1)],
                                kt_sb[pr][hp, i * 128 : (i + 1) * 128],
                                qt_sb[pr][hp, j * 512 + lo : (j + 1) * 512],
                                start=True,
                                stop=True,
                            )
                        et = etp.tile(
                            [128, 1024], bf16, name=f"et{pr}_{j}_{i}", tag="et"
                        )
                        if lo:
                            nc.scalar.activation(
                                et3(et)[:, :, lo:], et3(pss)[:, :, lo:],
                                Exp, scale=0.125,
                            )
                        else:
                            nc.scalar.activation(et[:], pss[:], Exp, scale=0.125)
                        if diag:
                            # zero the masked triangle of the straddling block
                            nc.vector.tensor_mul(
                                et3(et)[:, :, lo : lo + 128],
                                et3(et)[:, :, lo : lo + 128],
                                tri3,
                            )
                        if i == 1:
                            # lazy normalizes of previous block(s): emitted
                            # BEFORE this block's first attnV so their ps_out
                            # reads precede its overwrite in program order
                            # (psO has a single buffer).
                            for pn in pending_norms:
                                emit_normalize(*pn)
                                if pn[0] == 1:
                                    for mm in range(4 * pn[1], 4 * pn[1] + 4):
                                        ready_out.append((mm, 0))
                                        ready_out.append((mm, 1))
                            pending_norms.clear()
                        if prev_et is not None:
                            for hh in range(2):
                                nc.tensor.matmul(
                                    ps_out[:, 512 * hh + prev_lo : 512 * (hh + 1)],
                                    v_sb[prev_i][:, (2 * pr + hh) * 65 : (2 * pr + hh + 1) * 65],
                                    prev_et[:, 512 * hh + prev_lo : 512 * (hh + 1)],
                                    start=(prev_i == 0),
                                    stop=(prev_i == n_i - 1),
                                    skip_group_check=True,
                                )
                        prev_et, prev_i, prev_lo = et, i, lo
                        if i >= 2:
                            # one background PE unit per iteration: spreads
                            # deferred projections (pr0) and outproj chunks
                            # (pr1) so the exp pipeline never starves
                            if fillers:
                                pop_filler()
                            elif ready_out:
                                emit_outproj_chunk(*ready_out.pop(0))
                    for hh in range(2):
                        nc.tensor.matmul(
                            ps_out[:, 512 * hh + prev_lo : 512 * (hh + 1)],
                            v_sb[n_i - 1][:, (2 * pr + hh) * 65 : (2 * pr + hh + 1) * 65],
                            prev_et[:, 512 * hh + prev_lo : 512 * (hh + 1)],
                            start=(n_i - 1 == 0),
                            stop=True,
                            skip_group_check=True,
                        )
                    pending_norms.append((pr, j, ps_out))
            # drain: pr1 ends on j=0
            for pn in pending_norms:
                emit_normalize(*pn)
                if pn[0] == 1:
                    for mm in range(4 * pn[1], 4 * pn[1] + 4):
                        ready_out.append((mm, 0))
                        ready_out.append((mm, 1))
            pending_norms.clear()
            for ch in ready_out:
                emit_outproj_chunk(*ch)
            ready_out.clear()

    nc.compile()
    return nc


def _get_program():
    if "nc" not in _PROG_CACHE:
        _PROG_CACHE["nc"] = _build_program()
    return _PROG_CACHE["nc"]


def _host_prep(query, key, value, mask, w_q, w_k, w_v, w_o):
    import ml_dtypes

    bf = ml_dtypes.bfloat16
    query = np.asarray(query, dtype=np.float32)
    key = np.asarray(key, dtype=np.float32)
    value = np.asarray(value, dtype=np.float32)
    w_q = np.asarray(w_q, dtype=np.float32)
    w_k = np.asarray(w_k, dtype=np.float32)
    w_v = np.asarray(w_v, dtype=np.float32)
    w_o = np.asarray(w_o, dtype=np.float32)
    m = np.asarray(mask).reshape(S, S).astype(bool)

    # The kernel's block-skip structure assumes the standard causal mask.
    expected = np.triu(np.ones((S, S), dtype=bool), k=1)
    if not np.array_equal(m, expected):
        raise NotImplementedError("kernel specialized for causal (triu, k=1) mask")

    def tile_x(xT):  # [1024, 2048] -> [128, 8*2048] (e-tiles side by side)
        return np.ascontiguousarray(
            xT.reshape(ET, 128, S).transpose(1, 0, 2).reshape(128, ET * S).astype(bf)
        )

    def tile_w(w_rows):  # [256, 1024] slice -> [128, 8*256]
        t = w_rows.T.reshape(ET, 128, 256).transpose(1, 0, 2).reshape(128, ET * 256)
        return np.ascontiguousarray(t.astype(bf))

    xt = {}
    for b in range(B):
        xt[("q", b)] = tile_x(query[b].T)
        xt[("k", b)] = tile_x(key[b].T)
        xt[("v", b)] = tile_x(value[b].T)

    in_maps = []
    for c in range(N_CORES):
        b = c // 4
        hb = (c % 4) * HPC
        rs = slice(hb * D_K, (hb + HPC) * D_K)
        in_maps.append(
            {
                "xq": xt[("q", b)],
                "xk": xt[("k", b)],
                "xv": xt[("v", b)],
                "wq": tile_w(w_q[rs, :]),
                "wk": tile_w(w_k[rs, :]),
                "wv": tile_w(w_v[rs, :]),
                "wo": np.ascontiguousarray(w_o[:, rs].T.astype(bf)),
            }
        )
    return in_maps


def kernel(query, key, value, mask, w_q, w_k, w_v, w_o):
    from concourse.bass_utils import run_bass_kernel_spmd

    in_maps = _host_prep(query, key, value, mask, w_q, w_k, w_v, w_o)
    nc = _get_program()
    res = run_bass_kernel_spmd(nc, in_maps, list(range(N_CORES)))
    out = np.zeros((B, S, D_MODEL), dtype=np.float32)
    for c in range(N_CORES):
        out[c // 4] += res.results[c]["y"].astype(np.float32)
    return out


# revision 47
# speedup vs baseline: 1.0263x; 1.0263x over previous
"""Multi-head causal attention (B=2, S=2048, D=1024, H=16) on 8 TRN2 NeuronCores.

Sharding: batch*head parallel. Core c handles batch b = c//4 and the 4
heads h in [4*(c%4), 4*(c%4)+4). Each core computes its heads' Q/K/V
projections (column-parallel), causal softmax attention, and its partial
row-parallel output projection; the host sums the 4 partial outputs per
batch (the AllReduce of row-parallel tensor parallelism).

On-device layout: everything is kept "transposed" (feature-major) so
every matmul contracts along the partition dimension:
  scoresT[k,q] = K Q^T      (per head, 128-row k-tiles x 512-col q-tiles)
  P^T = exp(scoresT/8)      (diagonal blocks column-restricted; the
                             straddling 128x128 triangle is zeroed by a
                             DVE multiply with a 0/1 triangle tile)
  outT[d,q]   = sum_k V[k,d] P^T[k,q]   (PSUM-accumulated over k-tiles)
  sums[q]     = sum_k P^T[k,q]          (ones-vector matmul, col-packed)
  y[q,e]     += sum_hd outT_norm[hd,q] * w_oT[hd,e]
Softmax skips the max-subtraction: scores ~ N(0,1), so exp never
overflows fp32; fully-masked columns are simply never computed.

x / q,k,v weights / attention / outproj all run in bf16 (measured
~3.5e-3 max rel err vs the f32 reference, tolerance 2e-2). x is
host-pre-tiled to [128, 8*2048] (feature-major e-tiles side by side) and
held resident in SBUF, so Q/K/V projections all read the same resident
tiles (no strided V re-load). Constant tiles (ones / 0-1 triangle) are
generated on the idle GPSIMD engine (iota / affine_select) instead of
DMAed. Input DMAs are split ~0.5MB per queue and issued from both the
SP and Activation hardware DGE sequencers to halve issue serialization.

The PE HAM clock gate starts at half clock and only reaches full clock
after ~3.4us of gapless PE activity; a dummy warm-up pack (results never
read) runs while the x DMAs stream.
"""

import numpy as np

D_MODEL = 1024
N_HEADS = 16
D_K = 64
B, S = 2, 2048
N_CORES = 8
HPC = 4            # heads per core
KT = S // 128      # 16 k-tiles
QT = S // 512      # 4 q-tiles
ET = D_MODEL // 128  # 8 e-tiles (contraction tiles for projections)

_PROG_CACHE = {}


def _build_program():
    import concourse.bacc as bacc_mod
    import concourse.mybir as mybir
    import concourse.tile as tile

    f32 = mybir.dt.float32
    f32r = mybir.dt.float32r
    bf16 = mybir.dt.bfloat16
    Exp = mybir.ActivationFunctionType.Exp

    nc = bacc_mod.Bacc(
        "TRN2", target_bir_lowering=False, debug=False, num_devices=N_CORES
    )

    xq = nc.dram_tensor("xq", [128, ET * S], bf16, kind="ExternalInput").ap()
    xk = nc.dram_tensor("xk", [128, ET * S], bf16, kind="ExternalInput").ap()
    xv = nc.dram_tensor("xv", [128, ET * S], bf16, kind="ExternalInput").ap()
    wq = nc.dram_tensor("wq", [128, ET * 256], bf16, kind="ExternalInput").ap()
    wk = nc.dram_tensor("wk", [128, ET * 256], bf16, kind="ExternalInput").ap()
    wv = nc.dram_tensor("wv", [128, ET * 256], bf16, kind="ExternalInput").ap()
    wo = nc.dram_tensor("wo", [256, D_MODEL], bf16, kind="ExternalInput").ap()
    y = nc.dram_tensor("y", [S, D_MODEL], bf16, kind="ExternalOutput").ap()

    with (
        tile.TileContext(nc) as tc,
        nc.allow_low_precision("bf16 attention"),
        tc.tile_pool(name="persist", bufs=1) as pp,
    ):
        # ---- persistent SBUF tiles ----
        def persist(shape, dtype, name):
            return pp.tile(shape, dtype, name=name, tag=name)

        xq_sb = persist([128, ET * S], bf16, "xq_sb")
        xk_sb = persist([128, ET * S], bf16, "xk_sb")
        xv_sb = persist([128, ET * S], bf16, "xv_sb")
        wq_sb = persist([128, ET * 256], bf16, "wq_sb")
        wk_sb = persist([128, ET * 256], bf16, "wk_sb")
        wv_sb = persist([128, ET * 256], bf16, "wv_sb")
        wo_sb = [persist([128, D_MODEL], bf16, f"wo_sb{p}") for p in range(2)]
        gen_sb = persist([128, 512], bf16, "gen_sb")     # all-ones (iota)
        tri_sb = persist([128, 256], bf16, "tri_sb")     # 0/1 upper-tri x2
        onesf_sb = persist([128, 64], f32r, "onesf_sb")  # ones (bc lhsT)
        qt_sb = [persist([128, S], bf16, f"qt_sb{p}") for p in range(2)]
        kt_sb = [persist([128, S], bf16, f"kt_sb{p}") for p in range(2)]
        v_sb = [persist([128, 260], bf16, f"v_sb{i}") for i in range(KT)]
        outt_sb = [persist([128, S], bf16, f"outt_sb{p}") for p in range(2)]

        # ---- GPSIMD-generated constants (no DMA dependencies) ----
        nc.gpsimd.iota(
            gen_sb[:], pattern=[[0, 512]], base=1, channel_multiplier=0,
            allow_small_or_imprecise_dtypes=True,
        )
        nc.vector.tensor_copy(onesf_sb[:], gen_sb[:, 0:64])
        # tri[p, a*128 + c] = 1.0 if p <= c else 0.0  (keep where col-row >= 0)
        nc.gpsimd.affine_select(
            tri_sb[:].rearrange("p (a c) -> p a c", a=2),
            gen_sb[:, 0:256].rearrange("p (a c) -> p a c", a=2),
            pattern=[[0, 2], [1, 128]],
            compare_op=mybir.AluOpType.is_ge,
            fill=0.0,
            base=0,
            channel_multiplier=-1,
        )

        # ---- input DMAs ----
        # Weights ride the Activation engine's DGE queues (early, small, and
        # done before the first exp needs the Act sequencer). The x stream
        # is SP-issued in k-column chunks, xv first within each chunk so
        # the V/Q/K projections of chunk 0 can start ~20us in while later
        # chunks land.
        for w_dram, w_tile in ((wq, wq_sb), (wk, wk_sb), (wv, wv_sb)):
            for h in range(2):
                cs = slice(h * ET * 128, (h + 1) * ET * 128)
                nc.scalar.dma_start(out=w_tile[:, cs], in_=w_dram[:, cs])
        for p in range(2):
            nc.scalar.dma_start(
                out=wo_sb[p][:], in_=wo[p * 128 : (p + 1) * 128, :]
            )

        def kchunk(ap, kc, eh):
            return ap.rearrange("p (t k) -> p t k", t=ET)[
                :, eh * 2 : (eh + 1) * 2, kc * 512 : (kc + 1) * 512
            ]

        for kc in range(4):
            for x_dram, x_sb in ((xv, xv_sb), (xq, xq_sb), (xk, xk_sb)):
                for eh in range(4):
                    nc.sync.dma_start(
                        out=kchunk(x_sb[:], kc, eh), in_=kchunk(x_dram, kc, eh)
                    )

        # ---- phase C+D: attention with interleaved output projection ----
        # One head-pair per pass (pr = 0, 1). Per (pr, j): score tiles are
        # [128, 1024] head-pair PSUM tiles; ONE exp per round (column-
        # restricted on diagonal blocks). attnV accumulates into a
        # [65, 1024] pair tile (row 64 = sum of exp via the ones column of
        # v_sb). Normalization of q-block j-1 is emitted lazily inside
        # block j; a staging copy releases its ps_out PSUM early so the
        # next block's attnV is not blocked. The output projection of
        # block j-1 runs as dense filler inside the pr=1 pass.
        with (
            tc.tile_pool(name="psS", bufs=2, space="PSUM") as psS,
            tc.tile_pool(name="psO", bufs=1, space="PSUM") as psO,
            tc.tile_pool(name="psP", bufs=2, space="PSUM") as psP,
            tc.tile_pool(name="et", bufs=6) as etp,
            tc.tile_pool(name="bcsb", bufs=3) as bcp,
            tc.tile_pool(name="rcsb", bufs=3) as rcp,
            tc.tile_pool(name="ysb", bufs=3) as ysbp,
        ):
            tri3 = tri_sb[:].rearrange("p (a c) -> p a c", a=2)

            # constant exp-sum ones columns of the V tiles (written once)
            for i in range(KT):
                nc.vector.tensor_copy(
                    v_sb[i][:].rearrange("p (h c) -> p h c", c=65)[:, :, 64:65],
                    gen_sb[:, 0:4].rearrange("p (h c) -> p h c", c=1),
                )

            def et3(t):
                return t[:].rearrange("p (h q) -> p h q", h=2)

            # ---- chunked projections (shared [128,512] PSUM ring) ----
            def emit_v_tile(i):
                psv = psP.tile([128, 512], f32, name=f"psv_{i}", tag="pp")
                for e in range(ET):
                    nc.tensor.matmul(
                        psv[:, 0:256],
                        xv_sb[:, e * S + i * 128 : e * S + (i + 1) * 128],
                        wv_sb[:, e * 256 : (e + 1) * 256],
                        start=(e == 0),
                        stop=(e == ET - 1),
                    )
                nc.vector.tensor_copy(
                    v_sb[i][:].rearrange("p (h c) -> p h c", c=65)[:, :, 0:64],
                    psv[:, 0:256].rearrange("p (h d) -> p h d", d=64),
                )

            def emit_qk_chunk(which, m, kc):
                x_sb, w_tile, dst = (
                    (xq_sb, wq_sb, qt_sb) if which == "q" else (xk_sb, wk_sb, kt_sb)
                )
                psc = psP.tile(
                    [128, 512], f32, name=f"psc_{which}{m}_{kc}", tag="pp"
                )
                for e in range(ET):
                    nc.tensor.matmul(
                        psc[:],
                        w_tile[:, e * 256 + m * 128 : e * 256 + (m + 1) * 128],
                        x_sb[:, e * S + kc * 512 : e * S + (kc + 1) * 512],
                        start=(e == 0),
                        stop=(e == ET - 1),
                    )
                nc.vector.tensor_copy(dst[m][:, kc * 512 : (kc + 1) * 512], psc[:])

            # PE warm-up: dummy matmuls (results never read) walk the HAM
            # clock gate up to full speed before the first projections.
            wt = psP.tile([128, 512], f32, name="warm", tag="pp")
            for w in range(20):
                nc.tensor.matmul(
                    wt[:], gen_sb[:, 0:128], gen_sb[:, 0:512],
                    start=True, stop=True,
                )

            # block-j0 prologue; later chunks go through the filler queue
            for i in range(4):
                emit_v_tile(i)
            emit_qk_chunk("q", 0, 0)
            emit_qk_chunk("k", 0, 0)
            fillers = []  # (needed_by_block, closure)
            for kc in range(1, 4):
                for i in range(4 * kc, 4 * kc + 4):
                    fillers.append((kc, ("v", i)))
                fillers.append((kc, ("q", 0, kc)))
                fillers.append((kc, ("k", 0, kc)))
            for kc in range(4):
                fillers.append((4, ("q", 1, kc)))
                fillers.append((4, ("k", 1, kc)))

            def pop_filler():
                kind = fillers.pop(0)[1]
                if kind[0] == "v":
                    emit_v_tile(kind[1])
                else:
                    emit_qk_chunk(*kind)

            def emit_outproj_chunk(m, n):
                psy = psP.tile([128, 512], f32, name=f"psy_{m}_{n}", tag="pp")
                for p in range(2):
                    nc.tensor.matmul(
                        psy[:],
                        outt_sb[p][:, m * 128 : (m + 1) * 128],
                        wo_sb[p][:, n * 512 : (n + 1) * 512],
                        start=(p == 0),
                        stop=(p == 1),
                    )
                y_sb = ysbp.tile([128, 512], bf16, name=f"y_sb_{m}_{n}", tag="ysb")
                nc.vector.tensor_copy(y_sb[:], psy[:])
                for q in range(2):
                    nc.sync.dma_start(
                        out=y[
                            m * 128 : (m + 1) * 128,
                            n * 512 + q * 256 : n * 512 + (q + 1) * 256,
                        ],
                        in_=y_sb[:, q * 256 : (q + 1) * 256],
                    )

            def emit_normalize(pr, jj, ps_out_prev):
                qsj = slice(jj * 512, (jj + 1) * 512)
                ssb = rcp.tile([33, 512], f32, name=f"ssb_{pr}_{jj}", tag="ssb")
                for hh in range(2):
                    nc.vector.tensor_copy(
                        ssb[32 * hh : 32 * hh + 1, :],
                        ps_out_prev[64:65, 512 * hh : 512 * (hh + 1)],
                    )
                # staging copy releases ps_out for the next block's attnV
                stg = rcp.tile([64, 1024], f32, name=f"stg_{pr}_{jj}", tag="stg")
                nc.vector.tensor_copy(stg[:], ps_out_prev[0:64, :])
                rc32 = rcp.tile([33, 512], f32, name=f"rc32_{pr}_{jj}", tag="rc32")
                nc.vector.reciprocal_approx_fast(out=rc32[:], in_=ssb[:])
                rc = rcp.tile([33, 512], f32r, name=f"rc_{pr}_{jj}", tag="rc")
                nc.vector.tensor_copy(rc[:], rc32[:])
                for hh in range(2):
                    bch = psP.tile(
                        [128, 512], f32, name=f"ps_bc_{pr}_{jj}_{hh}", tag="pp"
                    )
                    nc.tensor.matmul(
                        bch[0:64, :],
                        onesf_sb[32 * hh : 32 * hh + 1, 0:64],
                        rc[32 * hh : 32 * hh + 1, :],
                        start=True,
                        stop=True,
                        tile_position=(32 * hh, 0),
                    )
                    bc_sb = bcp.tile(
                        [64, 512], f32, name=f"bc_sb_{pr}_{jj}_{hh}", tag="bc"
                    )
                    nc.vector.tensor_copy(bc_sb[:], bch[0:64, :])
                    nc.vector.tensor_mul(
                        outt_sb[pr][64 * hh : 64 * hh + 64, qsj],
                        stg[:, 512 * hh : 512 * (hh + 1)],
                        bc_sb[:],
                    )

            pending_norms = []  # (pr, j, ps_out) awaiting lazy normalize
            ready_out = []      # j's with both norms done, awaiting outproj
            for pr in range(2):
                # ascending j in BOTH passes: pr0 tracks the k-chunked DMA
                # arrival; pr1's early (small) blocks finish their
                # normalizes first so outproj chunks drip out through the
                # larger later blocks instead of piling into a serial drain.
                j_order = range(QT)
                for j in j_order:
                    n_i = 4 * j + 4
                    # projections this block depends on (flush leftovers)
                    lim = j if pr == 0 else 5
                    while fillers and fillers[0][0] <= lim:
                        pop_filler()
                    ps_out = psO.tile(
                        [65, 1024], f32, name=f"ps_out_{pr}_{j}", tag="o"
                    )
                    prev_et = None
                    prev_i = -1
                    prev_lo = 0
                    for i in range(n_i):
                        diag = i >= 4 * j
                        r = i - 4 * j
                        lo = 128 * r if diag else 0
                        pss = psS.tile(
                            [128, 1024], f32, name=f"ps_s{pr}_{j}_{i}", tag="s"
                        )
                        for hh in range(2):
                            hp = slice(64 * hh, 64 * hh + 64)
                            nc.tensor.matmul(
                                pss[:, 512 * hh + lo : 512 * (hh + 1)],
                                kt_sb[pr][hp, i * 128 : (i + 1) * 128],
                                qt_sb[pr][hp, j * 512 + lo : (j + 1) * 512],
                                start=True,
                                stop=True,
                            )
                        et = etp.tile(
                            [128, 1024], bf16, name=f"et{pr}_{j}_{i}", tag="et"
                        )
                        if lo:
                            nc.scalar.activation(
                                et3(et)[:, :, lo:], et3(pss)[:, :, lo:],
                                Exp, scale=0.125,
                            )
                        else:
                            nc.scalar.activation(et[:], pss[:], Exp, scale=0.125)
                        if diag:
                            # zero the masked triangle of the straddling block
                            nc.vector.tensor_mul(
                                et3(et)[:, :, lo : lo + 128],
                                et3(et)[:, :, lo : lo + 128],
                                tri3,
                            )
                        if i == 1:
                            # lazy normalizes of previous block(s): emitted
                            # BEFORE this block's first attnV so their ps_out
                            # reads precede its overwrite in program order
                            # (psO has a single buffer).
                            for pn in pending_norms:
                                emit_normalize(*pn)
                                if pn[0] == 1:
                                    for mm in range(4 * pn[1], 4 * pn[1] + 4):
                                        ready_out.append((mm, 0))
                                        ready_out.append((mm, 1))
                            pending_norms.clear()
                        if prev_et is not None:
                            for hh in range(2):
                                nc.tensor.matmul(
                                    ps_out[:, 512 * hh + prev_lo : 512 * (hh + 1)],
                                    v_sb[prev_i][:, (2 * pr + hh) * 65 : (2 * pr + hh + 1) * 65],
                                    prev_et[:, 512 * hh + prev_lo : 512 * (hh + 1)],
                                    start=(prev_i == 0),
                                    stop=(prev_i == n_i - 1),
                                    skip_group_check=True,
                                )
                        prev_et, prev_i, prev_lo = et, i, lo
                        if i >= 2:
                            # one background PE unit per iteration: spreads
                            # deferred projections (pr0) and outproj chunks
                            # (pr1) so the exp pipeline never starves
                            if fillers:
                                pop_filler()
                            elif ready_out:
                                emit_outproj_chunk(*ready_out.pop(0))
                    for hh in range(2):
                        nc.tensor.matmul(
                            ps_out[:, 512 * hh + prev_lo : 512 * (hh + 1)],
                            v_sb[n_i - 1][:, (2 * pr + hh) * 65 : (2 * pr + hh + 1) * 65],
                            prev_et[:, 512 * hh + prev_lo : 512 * (hh + 1)],
                            start=(n_i - 1 == 0),
                            stop=True,
                            skip_group_check=True,
                        )
                    pending_norms.append((pr, j, ps_out))
            # drain: pr1 ends on j=0
            for pn in pending_norms:
                emit_normalize(*pn)
                if pn[0] == 1:
                    for mm in range(4 * pn[1], 4 * pn[1] + 4):
                        ready_out.append((mm, 0))
                        ready_out.append((mm, 1))
            pending_norms.clear()
            for ch in ready_out:
                emit_outproj_chunk(*ch)
            ready_out.clear()

    nc.compile()
    return nc


def _get_program():
    if "nc" not in _PROG_CACHE:
        _PROG_CACHE["nc"] = _build_program()
    return _PROG_CACHE["nc"]


def _host_prep(query, key, value, mask, w_q, w_k, w_v, w_o):
    import ml_dtypes

    bf = ml_dtypes.bfloat16
    query = np.asarray(query, dtype=np.float32)
    key = np.asarray(key, dtype=np.float32)
    value = np.asarray(value, dtype=np.float32)
    w_q = np.asarray(w_q, dtype=np.float32)
    w_k = np.asarray(w_k, dtype=np.float32)
    w_v = np.asarray(w_v, dtype=np.float32)
    w_o = np.asarray(w_o, dtype=np.float32)
    m = np.asarray(mask).reshape(S, S).astype(bool)

    # The kernel's block-skip structure assumes the standard causal mask.
    expected = np.triu(np.ones((S, S), dtype=bool), k=1)
    if not np.array_equal(m, expected):
        raise NotImplementedError("kernel specialized for causal (triu, k=1) mask")

    def tile_x(xT):  # [1024, 2048] -> [128, 8*2048] (e-tiles side by side)
        return np.ascontiguousarray(
            xT.reshape(ET, 128, S).transpose(1, 0, 2).reshape(128, ET * S).astype(bf)
        )

    def tile_w(w_rows):  # [256, 1024] slice -> [128, 8*256]
        t = w_rows.T.reshape(ET, 128, 256).transpose(1, 0, 2).reshape(128, ET * 256)
        return np.ascontiguousarray(t.astype(bf))

    xt = {}
    for b in range(B):
        xt[("q", b)] = tile_x(query[b].T)
        xt[("k", b)] = tile_x(key[b].T)
        xt[("v", b)] = tile_x(value[b].T)

    in_maps = []
    for c in range(N_CORES):
        b = c // 4
        hb = (c % 4) * HPC
        rs = slice(hb * D_K, (hb + HPC) * D_K)
        in_maps.append(
            {
                "xq": xt[("q", b)],
                "xk": xt[("k", b)],
                "xv": xt[("v", b)],
                "wq": tile_w(w_q[rs, :]),
                "wk": tile_w(w_k[rs, :]),
                "wv": tile_w(w_v[rs, :]),
                "wo": np.ascontiguousarray(w_o[:, rs].T.astype(bf)),
            }
        )
    return in_maps


def kernel(query, key, value, mask, w_q, w_k, w_v, w_o):
    from concourse.bass_utils import run_bass_kernel_spmd

    in_maps = _host_prep(query, key, value, mask, w_q, w_k, w_v, w_o)
    nc = _get_program()
    res = run_bass_kernel_spmd(nc, in_maps, list(range(N_CORES)))
    out = np.zeros((B, S, D_MODEL), dtype=np.float32)
    for c in range(N_CORES):
        out[c // 4] += res.results[c]["y"].astype(np.float32)
    return out
